# revision 3
# baseline (speedup 1.0000x reference)
"""Multi-head self-attention (B=4, S=2048, E=1024, H=16) + residual + layernorm
on 8 Trainium2 NeuronCores.

Sharding: data-parallel over batch (4) x tensor-parallel over heads (2-way),
one (batch, head-half) pair per core.  Each core computes Q/K/V projections
for its 8 heads, transposed-scores attention (softmax denominator obtained by
augmenting V with a ones column), its partial WO projection, then a pairwise
ReduceScatter sums the two head-halves and splits rows; each core finishes
residual + bias + layernorm for its half of the rows.

All matmuls run in bf16 (fp32 PSUM accumulation); measured end-to-end
numerics vs the fp32 reference: rel_l2 ~ 8e-5.
"""
import numpy as np
import ml_dtypes

B, S, E = 4, 2048, 1024
H, D = 16, 64
E_LOC = 512            # channels per core (8 heads x 64)
N_CORES = 8

_CACHE = {}


def _build_nc():
    import concourse.bass as bass
    import concourse.mybir as mybir
    import concourse.tile as tile
    from concourse import bacc

    F32 = mybir.dt.float32
    BF16 = mybir.dt.bfloat16
    AF = mybir.ActivationFunctionType

    nc = bacc.Bacc("TRN2", target_bir_lowering=False, debug=False,
                   num_devices=N_CORES)

    # ---- external inputs (per-core shards, host-prepared)
    xT = nc.declare_dram_parameter("xT", [E, S], BF16, isOutput=False)
    x_res = nc.declare_dram_parameter("x_res", [S // 2, E], F32, isOutput=False)
    wqT = nc.declare_dram_parameter("wqT", [E, E_LOC], BF16, isOutput=False)
    wkT = nc.declare_dram_parameter("wkT", [E, E_LOC], BF16, isOutput=False)
    wvT = nc.declare_dram_parameter("wvT", [E, E_LOC], BF16, isOutput=False)
    woT = nc.declare_dram_parameter("woT", [E_LOC, E], BF16, isOutput=False)
    bq = nc.declare_dram_parameter("bq", [128, 4], F32, isOutput=False)
    bk = nc.declare_dram_parameter("bk", [128, 4], F32, isOutput=False)
    bv_row = nc.declare_dram_parameter("bv_row", [1, E_LOC], F32, isOutput=False)
    mask_t = nc.declare_dram_parameter("mask_t", [128, 16], F32, isOutput=False)
    wo_b_row = nc.declare_dram_parameter("wo_b_row", [1, E], F32, isOutput=False)
    ln_w_row = nc.declare_dram_parameter("ln_w_row", [1, E], F32, isOutput=False)
    ln_b_row = nc.declare_dram_parameter("ln_b_row", [1, E], F32, isOutput=False)

    # ---- external output: this core's LN'd half of the rows
    out_half = nc.declare_dram_parameter("out_half", [S // 2, E], F32,
                                         isOutput=True)

    def bc_ap(param, n):
        # broadcast a [1, n] dram row across 128 partitions
        return bass.AP(tensor=param, offset=0, ap=[[0, 128], [1, n]])

    with tile.TileContext(nc) as tc:
        with tc.tile_pool(name="persist", bufs=1) as pp, \
             tc.tile_pool(name="psum", bufs=4, space="PSUM") as ps, \
             tc.tile_pool(name="dram", bufs=1, space="DRAM") as dram, \
             tc.tile_pool(name="small", bufs=3) as sp:

            # ---------- constants / broadcasts ----------
            wo_t = pp.tile([128, 4, E], BF16, tag="wo")
            nc.sync.dma_start(out=wo_t[:], in_=woT.ap().rearrange(
                "(mt p) eo -> p mt eo", p=128))
            bq_t = pp.tile([128, 4], F32, tag="bq")
            nc.sync.dma_start(out=bq_t[:], in_=bq.ap())
            bk_t = pp.tile([128, 4], F32, tag="bk")
            nc.sync.dma_start(out=bk_t[:], in_=bk.ap())
            mask_sb = pp.tile([128, 16], F32, tag="mask")
            nc.sync.dma_start(out=mask_sb[:], in_=mask_t.ap())
            bv_bc = pp.tile([128, E_LOC], F32, tag="bv_bc")
            nc.sync.dma_start(out=bv_bc[:], in_=bc_ap(bv_row, E_LOC))
            wob_bc = pp.tile([128, E], F32, tag="wob_bc")
            nc.sync.dma_start(out=wob_bc[:], in_=bc_ap(wo_b_row, E))
            lnw_bc = pp.tile([128, E], F32, tag="lnw_bc")
            nc.sync.dma_start(out=lnw_bc[:], in_=bc_ap(ln_w_row, E))
            lnb_bc = pp.tile([128, E], F32, tag="lnb_bc")
            nc.sync.dma_start(out=lnb_bc[:], in_=bc_ap(ln_b_row, E))
            ones_row = pp.tile([1, 64], BF16, tag="ones_row")
            nc.vector.memset(ones_row[:], 1.0)
            eps_t = pp.tile([128, 1], F32, tag="eps")
            nc.vector.memset(eps_t[:], 1e-12)

            # persistent activations
            q_t = pp.tile([128, 4, S], BF16, tag="Q")     # [p, mt, s]
            k_t = pp.tile([128, 4, S], BF16, tag="K")
            v_t = pp.tile([128, 16, 8, 65], BF16, tag="V")  # [s2p, s2t, h, d+1]
            ctx_t = pp.tile([128, 4, S], BF16, tag="ctx")   # [p(m), mt, s1]

            # ones column of the augmented V (softmax denominator trick)
            nc.vector.memset(v_t[:, :, :, 64:65], 1.0)

            dram_part = dram.tile([S, E], F32, tag="part")
            dram_rs = dram.tile([S // 2, E], F32, tag="rs")

            # ---------- phase 1: Q/K/V projections ----------
            with tc.tile_pool(name="w1", bufs=1) as w1:
                xT_t = w1.tile([128, 8, S], BF16, tag="xT")
                nc.sync.dma_start(out=xT_t[:], in_=xT.ap().rearrange(
                    "(kt p) s -> p kt s", p=128))
                wq_t = w1.tile([128, 8, E_LOC], BF16, tag="wq")
                nc.sync.dma_start(out=wq_t[:], in_=wqT.ap().rearrange(
                    "(kt p) m -> p kt m", p=128))
                wk_t = w1.tile([128, 8, E_LOC], BF16, tag="wk")
                nc.sync.dma_start(out=wk_t[:], in_=wkT.ap().rearrange(
                    "(kt p) m -> p kt m", p=128))
                wv_t = w1.tile([128, 8, E_LOC], BF16, tag="wv")
                nc.sync.dma_start(out=wv_t[:], in_=wvT.ap().rearrange(
                    "(kt p) m -> p kt m", p=128))

                # Q, K: out [m, s] (m-major, for scores lhsT/rhs)
                for w_src, b_src, dst in ((wq_t, bq_t, q_t), (wk_t, bk_t, k_t)):
                    for mt in range(4):
                        for sb in range(4):
                            p = ps.tile([128, 512], F32, tag="mm")
                            for kt in range(8):
                                nc.tensor.matmul(
                                    p[:],
                                    w_src[:, kt, mt * 128:(mt + 1) * 128],
                                    xT_t[:, kt, sb * 512:(sb + 1) * 512],
                                    start=(kt == 0), stop=(kt == 7))
                            nc.vector.tensor_scalar_add(
                                out=dst[:, mt, sb * 512:(sb + 1) * 512],
                                in0=p[:], scalar1=b_src[:, mt:mt + 1])

                # V: out [s2, m] (s2-major, augmented with ones column)
                for s2t in range(16):
                    p = ps.tile([128, 512], F32, tag="mm")
                    for kt in range(8):
                        nc.tensor.matmul(
                            p[:],
                            xT_t[:, kt, s2t * 128:(s2t + 1) * 128],
                            wv_t[:, kt, :],
                            start=(kt == 0), stop=(kt == 7))
                    nc.vector.tensor_add(
                        out=v_t[:, s2t, :, 0:64],
                        in0=p[:].rearrange("p (h d) -> p h d", h=8),
                        in1=bv_bc[:].rearrange("p (h d) -> p h d", h=8))

            # ---------- phase 2+3: attention + WO, per s1-block ----------
            with tc.tile_pool(name="attn", bufs=1) as ap_pool:
                for sb1 in range(4):
                    s1 = slice(sb1 * 512, (sb1 + 1) * 512)
                    for hm in range(4):
                        exp_tiles = [
                            ap_pool.tile([128, 16, 512], BF16, tag="exp",
                                         bufs=4, name=f"exp{sb1}_{hm}_0"),
                            ap_pool.tile([128, 16, 512], BF16, tag="exp",
                                         bufs=4, name=f"exp{sb1}_{hm}_1"),
                        ]
                        # transposed scores + exp, two heads row-packed
                        for s2t in range(16):
                            for idx, hp in enumerate((0, 64)):
                                p = ps.tile([128, 512], F32, tag="mm")
                                nc.tensor.matmul(
                                    p[:],
                                    k_t[hp:hp + 64, hm,
                                        s2t * 128:(s2t + 1) * 128],
                                    q_t[hp:hp + 64, hm, s1],
                                    start=True, stop=True,
                                    tile_position=(hp, 0))
                                nc.scalar.activation(
                                    out=exp_tiles[idx][:, s2t, :], in_=p[:],
                                    func=AF.Exp,
                                    bias=mask_sb[:, s2t:s2t + 1], scale=0.125)
                        # PV with ones-augmented V; normalize into ctx
                        for idx, hp in enumerate((0, 64)):
                            hl = hm * 2 + idx
                            pv = ps.tile([65, 512], F32, tag="pv", bufs=2)
                            for s2t in range(16):
                                nc.tensor.matmul(
                                    pv[:], v_t[:, s2t, hl, :],
                                    exp_tiles[idx][:, s2t, :],
                                    start=(s2t == 0), stop=(s2t == 15))
                            den = sp.tile([1, 512], BF16, tag="den")
                            nc.vector.tensor_copy(out=den[:], in_=pv[64:65, :])
                            bcp = ps.tile([64, 512], F32, tag="bc", bufs=2)
                            nc.tensor.matmul(bcp[:], ones_row[:], den[:],
                                             start=True, stop=True)
                            rec = sp.tile([64, 512], F32, tag="rec")
                            nc.vector.reciprocal(out=rec[:], in_=bcp[:])
                            nc.vector.tensor_mul(
                                out=ctx_t[hp:hp + 64, hm, s1],
                                in0=pv[0:64, :], in1=rec[:])

                    # WO partial for this s1 block -> dram
                    for st in range(sb1 * 4, sb1 * 4 + 4):
                        for eb in range(2):
                            p = ps.tile([128, 512], F32, tag="mm")
                            for mt in range(4):
                                nc.tensor.matmul(
                                    p[:],
                                    ctx_t[:, mt, st * 128:(st + 1) * 128],
                                    wo_t[:, mt, eb * 512:(eb + 1) * 512],
                                    start=(mt == 0), stop=(mt == 3))
                            osb = sp.tile([128, 512], F32, tag="osb")
                            nc.vector.tensor_copy(out=osb[:], in_=p[:])
                            nc.sync.dma_start(
                                out=dram_part[st * 128:(st + 1) * 128,
                                              eb * 512:(eb + 1) * 512],
                                in_=osb[:])

            # ---------- phase 4: pairwise ReduceScatter ----------
            nc.gpsimd.collective_compute(
                "ReduceScatter", mybir.AluOpType.add,
                replica_groups=[[0, 1], [2, 3], [4, 5], [6, 7]],
                ins=[dram_part[:].opt()],
                outs=[dram_rs[:].opt()])

            # ---------- phase 5: residual + bias + layernorm ----------
            with tc.tile_pool(name="epi", bufs=3) as ep:
                for t in range(8):
                    rows = slice(t * 128, (t + 1) * 128)
                    v = ep.tile([128, E], F32, tag="v")
                    nc.sync.dma_start(out=v[:], in_=dram_rs[rows, :])
                    xr = ep.tile([128, E], F32, tag="xr")
                    nc.sync.dma_start(out=xr[:], in_=x_res.ap()[rows, :])
                    nc.vector.tensor_add(out=v[:], in0=v[:], in1=xr[:])
                    nc.vector.tensor_add(out=v[:], in0=v[:], in1=wob_bc[:])
                    stats = ep.tile([128, 2, 6], F32, tag="stats")
                    nc.vector.bn_stats(out=stats[:, 0, :], in_=v[:, 0:512])
                    nc.vector.bn_stats(out=stats[:, 1, :], in_=v[:, 512:1024])
                    mv = ep.tile([128, 2], F32, tag="mv")
                    nc.vector.bn_aggr(out=mv[:], in_=stats[:])
                    rstd = ep.tile([128, 1], F32, tag="rstd")
                    nc.scalar.activation(out=rstd[:], in_=mv[:, 1:2],
                                         func=AF.Sqrt,
                                         bias=eps_t[:, 0:1], scale=1.0)
                    nc.vector.reciprocal(out=rstd[:], in_=rstd[:])
                    nc.vector.tensor_scalar(
                        out=v[:], in0=v[:],
                        scalar1=mv[:, 0:1], scalar2=rstd[:],
                        op0=mybir.AluOpType.subtract,
                        op1=mybir.AluOpType.mult)
                    nc.vector.tensor_mul(out=v[:], in0=v[:], in1=lnw_bc[:])
                    nc.vector.tensor_add(out=v[:], in0=v[:], in1=lnb_bc[:])
                    nc.sync.dma_start(out=out_half.ap()[rows, :], in_=v[:])

    nc.finalize()
    return nc


def _prepare_in_maps(inputs):
    bf = ml_dtypes.bfloat16
    f32 = np.float32
    x = np.ascontiguousarray(inputs["input_tensor"], dtype=f32)
    mask = np.ascontiguousarray(inputs["mask"], dtype=f32)
    in_maps = []
    for c in range(N_CORES):
        b, hc = divmod(c, 2)
        rows = slice(hc * E_LOC, (hc + 1) * E_LOC)
        m = {
            "xT": np.ascontiguousarray(x[b].T).astype(bf),
            "x_res": np.ascontiguousarray(
                x[b, hc * (S // 2):(hc + 1) * (S // 2)]),
            "wqT": np.ascontiguousarray(inputs["WQ_w"][rows].T).astype(bf),
            "wkT": np.ascontiguousarray(inputs["WK_w"][rows].T).astype(bf),
            "wvT": np.ascontiguousarray(inputs["WV_w"][rows].T).astype(bf),
            "woT": np.ascontiguousarray(inputs["WO_w"][:, rows].T).astype(bf),
            "bq": np.ascontiguousarray(
                np.asarray(inputs["WQ_b"], f32)[rows].reshape(4, 128).T),
            "bk": np.ascontiguousarray(
                np.asarray(inputs["WK_b"], f32)[rows].reshape(4, 128).T),
            "bv_row": np.asarray(inputs["WV_b"], f32)[rows].reshape(1, E_LOC),
            "mask_t": np.ascontiguousarray(
                mask[b, 0, 0].reshape(16, 128).T),
            "wo_b_row": np.asarray(inputs["WO_b"], f32).reshape(1, E),
            "ln_w_row": np.asarray(inputs["ln_w"], f32).reshape(1, E),
            "ln_b_row": np.asarray(inputs["ln_b"], f32).reshape(1, E),
        }
        in_maps.append({k: np.ascontiguousarray(v) for k, v in m.items()})
    return in_maps


def _run(inputs, trace=False):
    from concourse.bass_utils import run_bass_kernel_spmd

    if "nc" not in _CACHE:
        _CACHE["nc"] = _build_nc()
    in_maps = _prepare_in_maps(inputs)
    res = run_bass_kernel_spmd(_CACHE["nc"], in_maps, list(range(N_CORES)),
                               trace=trace)
    out = np.empty((B, S, E), np.float32)
    for c in range(N_CORES):
        b, hc = divmod(c, 2)
        out[b, hc * (S // 2):(hc + 1) * (S // 2)] = res.results[c]["out_half"]
    return out, res


def kernel(**inputs):
    out, _ = _run(inputs, trace=False)
    return out
